# revision 1
# baseline (speedup 1.0000x reference)
"""Trainium2 Bass kernel for nn_MmbeddingsEncoder (segment_reduce).

Strategy (data-parallel over 8 NeuronCores):
  - rows (N=1e6) sharded 8-way; each core runs the 2-layer MLP on its shard
    (bf16 stationary-weight matmuls on PE),
  - local segment sums+counts via ONE combined GPSIMD scatter_add stream:
    each 16-partition group (Q7 core) consumes its own index stream, so we
    pack {set0,set1} x {row-quarters A..D} into the 128 partitions
    (16 partitions per stream, 4 features per channel in d-slots, counts in
    slot 4).  That cuts the serial per-core index stream 8x vs. the naive
    layout (31.25k indices instead of 250k).
  - the four quarter-accumulators are summed exactly with a small fp32-PSUM
    matmul against a 0/1 constant,
  - fp32 ReduceScatter over the 8 cores (each core owns 1024 segments), the
    slot-packed layout is shipped through the collective and unpacked after
    with tiny constant matmuls,
  - small dense head (divide, projections, reparameterized sample) per
    q-shard; host concatenates the 8 output shards.

Host-side work is limited to data-independent layout/dtype transforms
(sharding, padding, transpose, int16 repack).
"""

import numpy as np
import ml_dtypes

from contextlib import ExitStack

from concourse import bass, mybir, tile, bacc
from concourse.bass_utils import run_bass_kernel_spmd
from concourse.masks import make_identity

BF16 = mybir.dt.bfloat16
F32 = mybir.dt.float32
I16 = mybir.dt.int16

# problem constants (hardcoded per contract)
N = 1_000_000
D_IN = 64
H0, H1 = 128, 64
Q = 8192
D = 16
N_CORES = 8

R = N // N_CORES              # rows per core = 125000
RQ = R // 4                   # rows per quarter = 31250
CHUNK = 2048                  # rows per quarter per scatter_add call
N_CHUNK = 16
QP = CHUNK * N_CHUNK          # padded rows per quarter = 32768
QS = Q // N_CORES             # q-shard per core = 1024
NSLOT = 6                     # d-slots: 4 features + count + pad

MM = 512                      # matmul free-dim slab


def build_program(n_cores=N_CORES, qp=QP, n_chunk=N_CHUNK, q=Q, qs=None):
    """Build the SPMD Bass program."""
    if qs is None:
        qs = q // n_cores
    chunk = qp // n_chunk
    nmm = chunk // MM

    nc = bacc.Bacc("TRN2", target_bir_lowering=False, debug=False,
                   num_devices=n_cores)

    # ---- I/O ----
    xyt = nc.dram_tensor("xyt", [D_IN + 1, 4 * qp], BF16, kind="ExternalInput")
    idsw = {(s, k): nc.dram_tensor(f"idsw{s}{k}", [16, qp // 16], I16,
                                   kind="ExternalInput")
            for s in range(2) for k in range(4)}
    w0 = nc.dram_tensor("w0", [D_IN + 1, H0], BF16, kind="ExternalInput")
    b0 = nc.dram_tensor("b0", [H0, 1], F32, kind="ExternalInput")
    w1s = [nc.dram_tensor(f"w1_{j}", [H0, 32], BF16, kind="ExternalInput")
           for j in range(4)]
    b1s = [nc.dram_tensor(f"b1_{j}", [64, 1], F32, kind="ExternalInput")
           for j in range(4)]
    sum8 = [nc.dram_tensor(f"sum8_{s}", [128, 16], BF16, kind="ExternalInput")
            for s in range(2)]
    wm = [nc.dram_tensor(f"wm{s}", [H1, D], F32, kind="ExternalInput") for s in range(2)]
    bm = [nc.dram_tensor(f"bm{s}", [D, 1], F32, kind="ExternalInput") for s in range(2)]
    wv = [nc.dram_tensor(f"wv{s}", [H1, D], F32, kind="ExternalInput") for s in range(2)]
    bv = [nc.dram_tensor(f"bv{s}", [D, 1], F32, kind="ExternalInput") for s in range(2)]
    epst = [nc.dram_tensor(f"epst{s}", [D, qs], F32, kind="ExternalInput")
            for s in range(2)]
    out = nc.dram_tensor("out", [6, qs, D], F32, kind="ExternalOutput")

    AF = mybir.ActivationFunctionType
    OP = mybir.AluOpType

    with tile.TileContext(nc) as tc, ExitStack() as ctx:
        const = ctx.enter_context(tc.tile_pool(name="const", bufs=1))
        mid = ExitStack()  # lives until after extraction
        acc_pool = mid.enter_context(tc.tile_pool(name="acc", bufs=1))
        ids_pool = mid.enter_context(tc.tile_pool(name="ids", bufs=1))
        phase1 = ExitStack()
        xy_pool = phase1.enter_context(tc.tile_pool(name="xy", bufs=2))
        ht_pool = phase1.enter_context(tc.tile_pool(name="ht", bufs=2))
        add_pool = phase1.enter_context(tc.tile_pool(name="addt", bufs=1))
        ps1 = phase1.enter_context(tc.tile_pool(name="ps1", bufs=2, space="PSUM"))
        ps2 = phase1.enter_context(tc.tile_pool(name="ps2", bufs=1, space="PSUM"))

        # ---- constants / weights ----
        w0t = const.tile([D_IN + 1, H0], BF16)
        nc.sync.dma_start(out=w0t[:], in_=w0[:, :])
        b0t = const.tile([H0, 1], F32)
        nc.sync.dma_start(out=b0t[:], in_=b0[:, :])
        w1t = [const.tile([H0, 32], BF16, name=f"w1t{j}") for j in range(4)]
        b1t4 = [const.tile([64, 1], F32, name=f"b1t4{j}") for j in range(4)]
        for j in range(4):
            nc.sync.dma_start(out=w1t[j][:], in_=w1s[j][:, :])
            nc.sync.dma_start(out=b1t4[j][:], in_=b1s[j][:, :])
        sum8t = [const.tile([128, 16], BF16, name=f"sum8t{s}") for s in range(2)]
        for s in range(2):
            nc.sync.dma_start(out=sum8t[s][:], in_=sum8[s][:, :])
        wmt = [const.tile([H1, D], F32, name=f"wmt{s}") for s in range(2)]
        wvt = [const.tile([H1, D], F32, name=f"wvt{s}") for s in range(2)]
        bmt = [const.tile([D, 1], F32, name=f"bmt{s}") for s in range(2)]
        bvt = [const.tile([D, 1], F32, name=f"bvt{s}") for s in range(2)]
        for s in range(2):
            nc.sync.dma_start(out=wmt[s][:], in_=wm[s][:, :])
            nc.sync.dma_start(out=wvt[s][:], in_=wv[s][:, :])
            nc.sync.dma_start(out=bmt[s][:], in_=bm[s][:, :])
            nc.sync.dma_start(out=bvt[s][:], in_=bv[s][:, :])
        epstt = [const.tile([D, qs], F32, name=f"epstt{s}") for s in range(2)]
        for s in range(2):
            nc.sync.dma_start(out=epstt[s][:], in_=epst[s][:, :])
        ones64 = const.tile([1, H1], F32)
        nc.vector.memset(ones64[:], 1.0)
        ident = const.tile([128, 128], F32)
        make_identity(nc, ident[:])
        # unpack matrices: spread[j][c, 4c+j] = 1
        spread = []
        for j in range(4):
            sp = const.tile([16, H1], F32, name=f"spread{j}")
            nc.vector.memset(sp[:], 0.0)
            nc.vector.tensor_copy(out=sp[:, j:H1:4], in_=ident[0:16, 0:16])
            spread.append(sp)

        # ---- index streams: partition group 4s+k <- (set s, quarter k) ----
        idst = ids_pool.tile([128, qp // 16], I16)
        for s in range(2):
            for k in range(4):
                p0 = 32 * k + 16 * s
                nc.sync.dma_start(out=idst[p0:p0 + 16, :], in_=idsw[(s, k)][:, :])

        # ---- accumulator (bf16) [128, q, 6]; partition 16*(4s+k)+c,
        #      channel c = features {4c..4c+3} in slots 0..3, count slot 4 ----
        acc = acc_pool.tile([128, q * NSLOT], BF16)
        nc.vector.memset(acc[:], 0.0)

        # ---- add tiles (manually double buffered; counts preset once) ----
        addts = [add_pool.tile([128, chunk * NSLOT], BF16, name=f"addtile{p}")
                 for p in range(2)]
        for p in range(2):
            nc.vector.memset(addts[p][:], 0.0)
            nc.vector.memset(addts[p][:, 4:chunk * NSLOT:NSLOT], 1.0)

        # ---- main loop (quarters processed together per matmul slab so the
        #      z1 -> addt copies run as 64-partition ops) ----
        for ci in range(n_chunk):
            addt = addts[ci % 2]
            xts = []
            for k in range(4):
                base = k * qp + ci * chunk
                xt = xy_pool.tile([D_IN + 1, chunk], BF16, name=f"xt{k}")
                nc.sync.dma_start(out=xt[:], in_=xyt[:, base:base + chunk])
                xts.append(xt)
            for mi in range(nmm):
                t0 = mi * MM
                o0 = NSLOT * t0
                hss = []
                for k in range(4):
                    hp_ = ps1.tile([H0, MM], F32)
                    nc.tensor.matmul(hp_[:], lhsT=w0t[:],
                                     rhs=xts[k][:, mi * MM:(mi + 1) * MM],
                                     start=True, stop=True)
                    hs = ht_pool.tile([H0, MM], BF16, name=f"hs{k}")
                    nc.scalar.activation(hs[:], hp_[:], AF.Relu, bias=b0t[:, :])
                    hss.append(hs)
                for jp in range(2):
                    # ZP_p holds quarters {2p,2p+1} x j-pair {2jp, 2jp+1}
                    zps = [ps2.tile([64, 2 * MM], F32, name=f"zp{p}")
                           for p in range(2)]
                    for k in range(4):
                        for jj in range(2):
                            j = 2 * jp + jj
                            nc.tensor.matmul(
                                zps[k // 2][32 * (k % 2):32 * (k % 2) + 32,
                                            jj * MM:(jj + 1) * MM],
                                lhsT=w1t[j][:], rhs=hss[k][:],
                                start=True, stop=True)
                    for p in range(2):
                        for jj in range(2):
                            j = 2 * jp + jj
                            src_ = zps[p][:, jj * MM:(jj + 1) * MM]
                            dst_ = addt[64 * p:64 * (p + 1),
                                        o0 + j:o0 + NSLOT * MM:NSLOT]
                            nc.scalar.activation(dst_, src_, AF.Relu,
                                                 bias=b1t4[j][:, :])
            nc.gpsimd.scatter_add(
                in_ap=acc[:, :],
                idxs_ap=idst[:, ci * (chunk // 16):(ci + 1) * (chunk // 16)],
                add_ap=addt[:, :],
                channels=128, num_elems=q, d=NSLOT, num_idxs=chunk)

        phase1.close()

        # ---- extraction (sum quarters via matmul) + reduce-scatter ----
        sx_pool = mid.enter_context(tc.tile_pool(name="sx", bufs=2))
        pse = mid.enter_context(tc.tile_pool(name="pse", bufs=2, space="PSUM"))
        rs_in = [nc.dram_tensor(f"rs_in{s}", [n_cores, 16, qs * NSLOT], F32,
                                kind="Internal") for s in range(2)]
        rs_out = [nc.dram_tensor(f"rs_out{s}", [16, qs * NSLOT], F32,
                                 kind="Internal") for s in range(2)]
        for s in range(2):
            for g in range(n_cores):
                ext = sx_pool.tile([16, qs * NSLOT], F32, tag="ext")
                for j in range(qs * NSLOT // MM):
                    ep = pse.tile([16, MM], F32, tag="ep")
                    nc.tensor.matmul(
                        ep[:], lhsT=sum8t[s][:],
                        rhs=acc[:, g * qs * NSLOT + j * MM:
                                g * qs * NSLOT + (j + 1) * MM],
                        start=True, stop=True)
                    if j % 3 < 2:
                        nc.vector.tensor_copy(out=ext[:, j * MM:(j + 1) * MM],
                                              in_=ep[:])
                    else:
                        nc.scalar.copy(out=ext[:, j * MM:(j + 1) * MM],
                                       in_=ep[:])
                nc.sync.dma_start(out=rs_in[s][g], in_=ext[:])
            nc.gpsimd.collective_compute(
                "ReduceScatter", OP.add,
                replica_groups=[list(range(n_cores))],
                ins=[rs_in[s][:, :, :]], outs=[rs_out[s][:, :]])
        mid.close()

        # ---- head on owned q-shard ----
        head_pool = ctx.enter_context(tc.tile_pool(name="head", bufs=1))
        psh = ctx.enter_context(tc.tile_pool(name="psh", bufs=1, space="PSUM"))
        projT = []
        sampT = []
        for s in range(2):
            pck = head_pool.tile([16, qs * NSLOT], F32, name=f"pck{s}")
            nc.sync.dma_start(out=pck[:], in_=rs_out[s][:, :])
            cl = head_pool.tile([1, qs], F32, tag="cl")
            nc.vector.tensor_scalar_max(cl[:], pck[0:1, 4:qs * NSLOT:NSLOT], 1.0)
            rec = head_pool.tile([1, qs], F32, tag="rec")
            nc.vector.reciprocal(rec[:], cl[:])
            recb = head_pool.tile([H1, qs], F32, tag="recb")
            for jj in range(0, qs, MM):
                rp_ = psh.tile([H1, MM], F32, tag="recp")
                nc.tensor.matmul(rp_[:], lhsT=ones64[:], rhs=rec[:, jj:jj + MM],
                                 start=True, stop=True)
                nc.vector.tensor_copy(out=recb[:, jj:jj + MM], in_=rp_[:])
            bt = head_pool.tile([H1, qs], F32, tag="bt")
            for jj in range(0, qs, MM):
                up = psh.tile([H1, MM], F32, tag="up")
                for j in range(4):
                    nc.tensor.matmul(
                        up[:], lhsT=spread[j][:],
                        rhs=pck[:, jj * NSLOT + j:(jj + MM) * NSLOT:NSLOT],
                        start=(j == 0), stop=(j == 3))
                nc.vector.tensor_tensor(out=bt[:, jj:jj + MM], in0=up[:],
                                        in1=recb[:, jj:jj + MM], op=OP.mult)
            mT = head_pool.tile([D, qs], F32, name=f"mT{s}")
            vT = head_pool.tile([D, qs], F32, name=f"vT{s}")
            for (wt, bt_, dst) in ((wmt[s], bmt[s], mT), (wvt[s], bvt[s], vT)):
                for jj in range(0, qs, MM):
                    pp = psh.tile([D, MM], F32, tag="proj")
                    nc.tensor.matmul(pp[:], lhsT=wt[:], rhs=bt[:, jj:jj + MM],
                                     start=True, stop=True)
                    nc.vector.tensor_scalar(out=dst[:, jj:jj + MM], in0=pp[:],
                                            scalar1=bt_[:, :], scalar2=None,
                                            op0=OP.add)
            projT.append((mT, vT))
            e = head_pool.tile([D, qs], F32, name=f"eT{s}")
            nc.scalar.activation(e[:], vT[:], AF.Exp, scale=0.5)
            sm = head_pool.tile([D, qs], F32, name=f"smT{s}")
            nc.vector.tensor_tensor(out=sm[:], in0=e[:], in1=epstt[s][:],
                                    op=OP.mult)
            nc.vector.tensor_tensor(out=sm[:], in0=sm[:], in1=mT[:], op=OP.add)
            sampT.append(sm)

        # ---- transpose back to natural layout + output ----
        slabs = [projT[0][0], projT[1][0], projT[0][1], projT[1][1],
                 sampT[0], sampT[1]]
        nt = qs // 128
        ost = head_pool.tile([128, 6 * nt * D], F32, tag="ost")
        for si_, src in enumerate(slabs):
            for t in range(nt):
                tp = psh.tile([128, D], F32, tag="otp")
                nc.tensor.transpose(tp[:], src[:, t * 128:(t + 1) * 128],
                                    ident[0:D, 0:D])
                o = (si_ * nt + t) * D
                nc.vector.tensor_copy(out=ost[:, o:o + D], in_=tp[:])
        for si_ in range(6):
            nc.sync.dma_start(
                out=out[si_].rearrange("(t p) d -> p t d", p=128),
                in_=ost[:, si_ * nt * D:(si_ + 1) * nt * D].rearrange(
                    "p (t d) -> p t d", d=D))

    nc.compile()
    return nc


_CACHE = {}


def _get_program():
    if "nc" not in _CACHE:
        _CACHE["nc"] = build_program()
    return _CACHE["nc"]


def _prep_inputs(X, y, z_ids0, z_ids1, W0, b0, W1, b1,
                 Wm0, bm0, Wv0, bv0, Wm1, bm1, Wv1, bv1, eps0, eps1,
                 n_cores=N_CORES, r=R, qp=QP, qs=QS):
    """Host-side data-independent prep: shard/pad/layout/dtype only."""
    bf16 = ml_dtypes.bfloat16
    rq = r // 4
    xy = np.concatenate([np.asarray(X), np.asarray(y)], axis=1)  # [N, 65]
    xyt_full = np.ascontiguousarray(xy.T.astype(bf16))           # [65, N]

    in_maps = []
    for c in range(n_cores):
        lo = c * r
        m = {}
        xt = np.zeros((D_IN + 1, 4 * qp), dtype=bf16)
        for k in range(4):
            n_k = rq if k < 3 else r - 3 * rq
            xt[:, k * qp:k * qp + n_k] = xyt_full[:, lo + k * rq:lo + k * rq + n_k]
        m["xyt"] = xt
        for s, ids in enumerate((z_ids0, z_ids1)):
            idc = np.asarray(ids[lo:lo + r]).astype(np.int16)
            for k in range(4):
                n_k = rq if k < 3 else r - 3 * rq
                idp = np.full((qp,), -1, dtype=np.int16)
                idp[:n_k] = idc[k * rq:k * rq + n_k]
                m[f"idsw{s}{k}"] = np.ascontiguousarray(
                    idp.reshape(qp // 16, 16).T)
        m["w0"] = np.asarray(W0).astype(bf16)
        m["b0"] = np.asarray(b0).astype(np.float32).reshape(H0, 1)
        W1np = np.asarray(W1).astype(bf16)
        b1np = np.asarray(b1).astype(np.float32)
        for j in range(4):
            wj = W1np[:, j::4]                      # [128, 16]
            m[f"w1_{j}"] = np.ascontiguousarray(np.hstack([wj, wj]))
            bj = b1np[j::4]
            m[f"b1_{j}"] = np.ascontiguousarray(np.tile(bj, 4).reshape(64, 1))
        for s in range(2):
            s8 = np.zeros((128, 16), dtype=bf16)
            for p in range(128):
                cc = p % 32 - 16 * s
                if 0 <= cc < 16:
                    s8[p, cc] = 1
            m[f"sum8_{s}"] = s8
        for s, (Wm, bm, Wv, bv, eps) in enumerate(
                ((Wm0, bm0, Wv0, bv0, eps0), (Wm1, bm1, Wv1, bv1, eps1))):
            m[f"wm{s}"] = np.asarray(Wm).astype(np.float32)
            m[f"bm{s}"] = np.asarray(bm).astype(np.float32).reshape(D, 1)
            m[f"wv{s}"] = np.asarray(Wv).astype(np.float32)
            m[f"bv{s}"] = np.asarray(bv).astype(np.float32).reshape(D, 1)
            m[f"epst{s}"] = np.ascontiguousarray(
                np.asarray(eps[c * qs:(c + 1) * qs]).astype(np.float32).T)
        in_maps.append(m)
    return in_maps


def kernel(**inputs):
    nc = _get_program()
    in_maps = _prep_inputs(**inputs)
    res = run_bass_kernel_spmd(nc, in_maps, core_ids=list(range(N_CORES)))
    shards = [res.results[c]["out"] for c in range(N_CORES)]
    return np.concatenate(shards, axis=1).astype(np.float32)


if __name__ == "__main__":
    nc = build_program()
    print("program built OK")

